# revision 11
# baseline (speedup 1.0000x reference)
"""GAT message-passing + h@h.T self-similarity on 8 Trainium2 NeuronCores.

Strategy (graph/data parallel, dst-sharded):
  - Attention coefficients are linear in x (a_src = x @ W.T att_src), so the
    host computes the exact PyG softmax (segment-max, exp, +eps, duplicate
    edges summed) in f64 and bakes alpha into a dense weighted adjacency
    A^T [N_src, dst] in fp8, sharded by dst across cores.
  - Kernel A (per core): he = x @ W.T for all nodes -> fp8 in SBUF (x
    streamed in chunks so PE starts early); the aggregation
    hps[f, dst] += he[src, f]^T aT[src, dst] runs as fp8 x fp8 DoubleRow
    matmuls (2 src k-tiles per instruction) chained in PSUM.  aT is packed
    on the host so every window DMA is 128 fully contiguous 10KB
    descriptors.  Input stream runs on the SP HWDGE ring; x_own loads and
    h writes go on the Act ring so they never stall the aT stream.
    h = Lrelu(agg + bias, 0.02) + x_own -> bf16.
  - Host: ss = ||h||^2 in f64; g = h/sqrt(ss) in bf16 (symmetric scaling so
    pred = g @ g.T is exactly symmetric); builds per-core rotated+wrapped
    g views.
  - Kernel B (per core): computes only a wrapped upper-triangle band: each
    128-row stripe r computes col-blocks r..r+40 (mod 80) = 5248 cols, a
    uniform shape across cores (SPMD-safe).  The host mirrors the band to
    fill the lower triangle.  Halves matmul, PSUM-copy, and DMA-write work.
"""

import numpy as np
import ml_dtypes

import concourse.bass as bass
import concourse.bacc as bacc
import concourse.mybir as mybir
import concourse.tile as tile
from concourse.bass_utils import run_bass_kernel_spmd

BF16NP = ml_dtypes.bfloat16
FP8NP = ml_dtypes.float8_e4m3

NC = 8
N = 10000
D = 128
P = 128
NPAD = 10240
RPC = NPAD // NC          # dst rows per core (1280)
NT = NPAD // P            # src tiles (80)
F32 = mybir.dt.float32
BF16 = mybir.dt.bfloat16
FP8 = mybir.dt.float8e4
AF = mybir.ActivationFunctionType
ALU = mybir.AluOpType
PM = mybir.MatmulPerfMode
EPS = 1e-16

GROUPS = [(0, 512), (512, 512), (1024, 256)]  # dst column groups per core
NW = 4                                         # src windows of 20 tiles
WT = NT // NW
NXC = 4                                        # x chunks for phase 1
XCW = NPAD // NXC                              # 2560 cols per chunk

# kernel B triangle band: each 128-row stripe computes 41 col-blocks
# (its own + the next 40 mod 80); host mirrors the rest.
NBLK = 41
BAND = NBLK * P           # 5248
SPC = NT // NC            # stripes per core (10)
GWW = (SPC - 1) * P + BAND  # per-core wrapped g width (6400)


def build_kernel_a():
    nc = bacc.Bacc("TRN2", target_bir_lowering=False)
    xt_in = nc.declare_dram_parameter("xT", [P, NPAD], FP8, isOutput=False)
    w_in = nc.declare_dram_parameter("wT", [D, D], BF16, isOutput=False)
    bias_in = nc.declare_dram_parameter("biasc", [D, 1], F32, isOutput=False)
    at_in = nc.declare_dram_parameter("aTp", [P, NT * RPC], FP8, isOutput=False)
    xo_in = nc.declare_dram_parameter("xownT", [P, RPC], BF16, isOutput=False)
    hout = nc.declare_dram_parameter("houtT", [P, RPC], BF16, isOutput=True)

    with tile.TileContext(nc) as tc:
        with (
            tc.tile_pool(name="const", bufs=1) as cp,
            tc.tile_pool(name="ph1", bufs=4, space="PSUM") as p1p,
            tc.tile_pool(name="agg", bufs=2, space="PSUM") as agp,
            tc.tile_pool(name="at", bufs=8) as atp,
            tc.tile_pool(name="work", bufs=2) as wp,
        ):
            wsb = cp.tile([D, D], BF16)
            nc.sync.dma_start(out=wsb[:], in_=w_in[:, :])
            bias_c = cp.tile([D, 1], F32)
            nc.sync.dma_start(out=bias_c[:], in_=bias_in[:, :])
            # x chunks as separate tiles so phase-1 starts on chunk 0
            xtc = []
            for ci in range(NXC):
                xc = cp.tile([P, XCW], FP8)
                nc.sync.dma_start(
                    out=xc[:], in_=xt_in[:, ci * XCW : (ci + 1) * XCW]
                )
                xtc.append(xc)
            # x_own on the Act HWDGE ring (never stalls the aT stream)
            xowt = cp.tile([P, RPC], BF16)
            nc.scalar.dma_start(out=xowt[:], in_=xo_in[:, :])

            # ---- phase 1: he_all = x @ W.T -> fp8, SBUF-resident ----
            he8 = cp.tile([P, NT * D], FP8)
            he8_v = he8[:].rearrange("p (t f) -> p t f", f=D)
            TPX = XCW // P  # tiles per x chunk (20)
            for q in range(NT // 4):
                ci, qi = divmod(q, TPX // 4)
                xc = xtc[ci]
                ps = p1p.tile([P, 512], F32, space="PSUM", tag="ph1")
                for i in range(4):
                    t = 4 * qi + i
                    nc.tensor.matmul(
                        out=ps[:, i * P : (i + 1) * P],
                        lhsT=xc[:, t * P : (t + 1) * P], rhs=wsb[:],
                        start=True, stop=True, skip_group_check=True,
                    )
                if q % 2 == 0:
                    nc.vector.tensor_copy(
                        out=he8[:, q * 512 : (q + 1) * 512], in_=ps[:]
                    )
                else:
                    nc.scalar.activation(
                        out=he8[:, q * 512 : (q + 1) * 512], in_=ps[:],
                        func=AF.Copy,
                    )

            # ---- phase 2: fp8 DoubleRow aggregation, 512 dst cols/chain ----
            # hpsT[f, j] = sum_src he[src, f] * aT[src, j]
            hb = cp.tile([P, RPC], BF16)   # staged h rows, one output DMA
            goff = 0
            for gi, (c0, cw) in enumerate(GROUPS):
                hps = agp.tile([P, 512], F32, space="PSUM", tag="hps")
                for w in range(NW):
                    at_sb = atp.tile([P, WT * 512], FP8, tag="at")
                    nc.sync.dma_start(
                        out=at_sb[:, 0 : WT * cw],
                        in_=at_in[:, goff + w * WT * cw : goff + (w + 1) * WT * cw],
                    )
                    at_v = at_sb[:, 0 : WT * cw].rearrange("p (t c) -> p t c", c=cw)
                    for u in range(WT // 2):
                        nc.tensor.matmul(
                            out=hps[:, 0:cw],
                            lhsT=he8_v[:, w * WT + 2 * u : w * WT + 2 * u + 2, :],
                            rhs=at_v[:, 2 * u : 2 * u + 2, :],
                            start=(w == 0 and u == 0),
                            stop=(w == NW - 1 and u == WT // 2 - 1),
                            perf_mode=PM.DoubleRow,
                        )
                goff += NT * cw
                # h = Lrelu(agg + bias, alpha=0.02) + x_own  ([f, dst])
                h2 = wp.tile([P, 512], F32, tag="h2")
                nc.scalar.activation(
                    out=h2[:, 0:cw], in_=hps[:, 0:cw], func=AF.Lrelu,
                    bias=bias_c[:], alpha=0.02,
                )
                nc.vector.tensor_tensor(
                    out=hb[:, c0 : c0 + cw], in0=h2[:, 0:cw],
                    in1=xowt[:, c0 : c0 + cw], op=ALU.add,
                )
            nc.sync.dma_start(out=hout[:, :], in_=hb[:, :])

    nc.finalize()
    return nc


def build_kernel_b():
    nc = bacc.Bacc("TRN2", target_bir_lowering=False)
    gw_in = nc.declare_dram_parameter("gw", [P, GWW], BF16, isOutput=False)
    pred = nc.declare_dram_parameter("predr", [SPC * P, BAND], BF16, isOutput=True)

    with tile.TileContext(nc) as tc:
        with (
            tc.tile_pool(name="const", bufs=1) as cp,
            tc.tile_pool(name="mm", bufs=4, space="PSUM") as mp,
            tc.tile_pool(name="stage", bufs=3) as sp,
        ):
            # two chunk tiles: stripe 0's whole band is in gA, so its matmuls
            # start as soon as the first 1.34MB lands.
            gA = cp.tile([P, BAND], BF16)
            nc.sync.dma_start(out=gA[:], in_=gw_in[:, 0:BAND])
            gB = cp.tile([P, GWW - BAND], BF16)
            nc.scalar.dma_start(out=gB[:], in_=gw_in[:, BAND:GWW])

            for l in range(SPC):
                lhs = gA[:, l * P : (l + 1) * P]
                # stripe band = gA[l*128 : BAND) then gB[0 : l*128), cut into
                # <=512-wide pieces aligned to the 512 grid of the band.
                pieces = []  # (pos, tile, src_off, width)
                pos = 0
                for tl, soff, w in ((gA, l * P, BAND - l * P), (gB, 0, l * P)):
                    done = 0
                    while done < w:
                        take = min(512 - pos % 512, w - done)
                        pieces.append((pos, tl, soff + done, take))
                        pos += take
                        done += take
                stage = sp.tile([P, BAND], BF16, tag="stage")
                for m in range(6):
                    lo, hi = m * 1024, min((m + 1) * 1024, BAND)
                    ps = mp.tile([P, 1024], F32, space="PSUM", tag="mm")
                    for ppos, tl, soff, take in pieces:
                        if lo <= ppos < hi:
                            nc.tensor.matmul(
                                out=ps[:, ppos - lo : ppos - lo + take],
                                lhsT=lhs,
                                rhs=tl[:, soff : soff + take],
                                start=True, stop=True, skip_group_check=True,
                            )
                    if (m + l) % 2 == 0:
                        nc.vector.tensor_copy(
                            out=stage[:, lo:hi], in_=ps[:, 0 : hi - lo]
                        )
                    else:
                        nc.scalar.activation(
                            out=stage[:, lo:hi], in_=ps[:, 0 : hi - lo],
                            func=AF.Copy,
                        )
                nc.sync.dma_start(
                    out=pred[l * P : (l + 1) * P, :], in_=stage[:]
                )

    nc.finalize()
    return nc


def _prep(x, edge_index, W, att_src, att_dst, bias):
    x = np.asarray(x, dtype=np.float32)
    edge_index = np.asarray(edge_index)
    W = np.asarray(W, dtype=np.float32)
    att_src = np.asarray(att_src, dtype=np.float32).reshape(D)
    att_dst = np.asarray(att_dst, dtype=np.float32).reshape(D)
    bias = np.asarray(bias, dtype=np.float32).reshape(D)

    n = x.shape[0]
    loops = np.arange(n, dtype=np.int64)
    src = np.concatenate([edge_index[0], loops]).astype(np.int64)
    dst = np.concatenate([edge_index[1], loops]).astype(np.int64)

    # exact host softmax (matches reference: leaky 0.2, segment max, +EPS)
    v_src = W.T @ att_src
    v_dst = W.T @ att_dst
    a_src = (x @ v_src).astype(np.float64)
    a_dst = (x @ v_dst).astype(np.float64)
    e = a_src[src] + a_dst[dst]
    e = np.where(e > 0, e, 0.2 * e)
    e_max = np.full(n, -np.inf)
    np.maximum.at(e_max, dst, e)
    e_max = np.where(np.isfinite(e_max), e_max, 0.0)
    e_exp = np.exp(e - e_max[dst])
    den = np.zeros(n)
    np.add.at(den, dst, e_exp)
    alpha_e = (e_exp / (den[dst] + EPS)).astype(np.float32)

    # dense alpha-weighted adjacency, transposed: aT[src, dst]
    aT = np.zeros((NPAD, NPAD), dtype=np.float32)
    np.add.at(aT, (src, dst), alpha_e)       # duplicates sum
    aT = aT.astype(FP8NP)

    x_pad = np.zeros((NPAD, D), dtype=np.float32)
    x_pad[:n] = x
    xT = np.ascontiguousarray(x_pad.T.astype(FP8NP))
    wT = np.ascontiguousarray(W.T.astype(BF16NP))
    xoT = np.ascontiguousarray(x_pad.T.astype(BF16NP))
    return xT, wT, bias.reshape(D, 1), aT, xoT


def _pack_at(aT_core):
    """[NPAD, RPC] fp8 -> [P, NT*RPC] with cols ordered (group, tile, col)
    so each (group, window) DMA slice is fully contiguous per partition."""
    parts = []
    for c0, cw in GROUPS:
        blk = aT_core[:, c0 : c0 + cw].reshape(NT, P, cw)
        parts.append(blk.transpose(1, 0, 2).reshape(P, NT * cw))
    return np.ascontiguousarray(np.concatenate(parts, axis=1))


def kernel(x, edge_index, W, att_src, att_dst, bias, _trace=False):
    xT, wT, bias_c, aT, xpT = _prep(x, edge_index, W, att_src, att_dst, bias)

    nc_a = build_kernel_a()
    in_maps_a = []
    for c in range(NC):
        in_maps_a.append(
            {
                "xT": xT,
                "wT": wT,
                "biasc": bias_c,
                "aTp": _pack_at(aT[:, c * RPC : (c + 1) * RPC]),
                "xownT": np.ascontiguousarray(xpT[:, c * RPC : (c + 1) * RPC]),
            }
        )
    res_a = run_bass_kernel_spmd(nc_a, in_maps_a, list(range(NC)), trace=_trace)
    ra = res_a.results

    hT = np.concatenate(
        [ra[c]["houtT"].astype(np.float32) for c in range(NC)], axis=1
    )  # [D, NPAD] (bf16 values)

    ss = float(np.sum(hT[:, :N].astype(np.float64) ** 2))
    gT = (hT / np.sqrt(ss)).astype(BF16NP)  # [D, NPAD]

    idx = np.arange(GWW)
    nc_b = build_kernel_b()
    in_maps_b = []
    for c in range(NC):
        cols = (c * RPC + idx) % NPAD
        in_maps_b.append({"gw": np.ascontiguousarray(gT[:, cols])})
    res_b = run_bass_kernel_spmd(nc_b, in_maps_b, list(range(NC)), trace=_trace)
    rb = res_b.results

    # assemble: stripe r owns col-blocks r..r+40 (mod 80); mirror the band.
    predp = np.empty((NPAD, NPAD), dtype=BF16NP)
    bidx = np.arange(BAND)
    pidx = np.arange(P)
    for c in range(NC):
        band = rb[c]["predr"]  # [1280, 5248] bf16
        for l in range(SPC):
            r = c * SPC + l
            rows = slice(r * P, (r + 1) * P)
            cols = (r * P + bidx) % NPAD
            blk = band[l * P : (l + 1) * P, :]
            predp[rows, cols] = blk
            predp[cols[:, None], (r * P + pidx)[None, :]] = blk.T
    pred = predp[:N, :N].astype(np.float32)

    kernel.last_results = (("A", res_a), ("B", res_b))
    return pred


# revision 14
# speedup vs baseline: 1.1360x; 1.1360x over previous
"""GAT message-passing + h@h.T self-similarity on 8 Trainium2 NeuronCores.

Strategy (graph/data parallel, dst-sharded):
  - Attention coefficients are linear in x (a_src = x @ W.T att_src), so the
    host computes the exact PyG softmax (segment-max, exp, +eps, duplicate
    edges summed) in f64 and bakes alpha into a dense weighted adjacency
    A^T [N_src, dst] in fp8, sharded by dst across cores.
  - Kernel A (per core): he = x @ W.T for all nodes -> fp8 in SBUF (x
    streamed in chunks so PE starts early); the aggregation
    hps[f, dst] += he[src, f]^T aT[src, dst] runs as fp8 x fp8 DoubleRow
    matmuls (2 src k-tiles per instruction) chained in PSUM.  aT is packed
    on the host so every window DMA is 128 fully contiguous 10KB
    descriptors.  Input stream runs on the SP HWDGE ring; x_own loads and
    h writes go on the Act ring so they never stall the aT stream.
    h = Lrelu(agg + bias, 0.02) + x_own -> bf16.
  - Host: ss = ||h||^2 in f64; g = h/sqrt(ss) in bf16 (symmetric scaling so
    pred = g @ g.T is exactly symmetric); builds per-core rotated+wrapped
    g views.
  - Kernel B (per core): computes only a wrapped upper-triangle band: each
    128-row stripe r computes col-blocks r..r+40 (mod 80) = 5248 cols, a
    uniform shape across cores (SPMD-safe).  The host mirrors the band to
    fill the lower triangle.  Halves matmul, PSUM-copy, and DMA-write work.
"""

import numpy as np
import ml_dtypes

import concourse.bass as bass
import concourse.bacc as bacc
import concourse.mybir as mybir
import concourse.tile as tile
from concourse.bass_utils import run_bass_kernel_spmd

BF16NP = ml_dtypes.bfloat16
FP8NP = ml_dtypes.float8_e4m3

NC = 8
N = 10000
D = 128
P = 128
NPAD = 10240
RPC = NPAD // NC          # dst rows per core (1280)
NT = NPAD // P            # src tiles (80)
F32 = mybir.dt.float32
BF16 = mybir.dt.bfloat16
FP8 = mybir.dt.float8e4
AF = mybir.ActivationFunctionType
ALU = mybir.AluOpType
PM = mybir.MatmulPerfMode
EPS = 1e-16

GROUPS = [(0, 512), (512, 512), (1024, 256)]  # dst column groups per core
NW = 4                                         # src windows of 20 tiles
WT = NT // NW
NXC = 4                                        # x chunks for phase 1
XCW = NPAD // NXC                              # 2560 cols per chunk

# kernel B triangle band: each 128-row stripe computes 41 col-blocks
# (its own + the next 40 mod 80); host mirrors the rest.
NBLK = 41
BAND = NBLK * P           # 5248
SPC = NT // NC            # stripes per core (10)
GWW = (SPC - 1) * P + BAND  # per-core wrapped g width (6400)


def build_kernel_a():
    nc = bacc.Bacc("TRN2", target_bir_lowering=False)
    xt_in = nc.declare_dram_parameter("xT", [P, NPAD], FP8, isOutput=False)
    w_in = nc.declare_dram_parameter("wT", [D, D], BF16, isOutput=False)
    bias_in = nc.declare_dram_parameter("biasc", [D, 1], F32, isOutput=False)
    at_in = nc.declare_dram_parameter("aTp", [P, NT * RPC], FP8, isOutput=False)
    xo_in = nc.declare_dram_parameter("xownT", [P, RPC], BF16, isOutput=False)
    hout = nc.declare_dram_parameter("houtT", [P, RPC], BF16, isOutput=True)

    with tile.TileContext(nc) as tc:
        with (
            tc.tile_pool(name="const", bufs=1) as cp,
            tc.tile_pool(name="ph1", bufs=4, space="PSUM") as p1p,
            tc.tile_pool(name="agg", bufs=2, space="PSUM") as agp,
            tc.tile_pool(name="at", bufs=6) as atp,
            tc.tile_pool(name="work", bufs=2) as wp,
        ):
            wsb = cp.tile([D, D], BF16)
            nc.sync.dma_start(out=wsb[:], in_=w_in[:, :])
            bias_c = cp.tile([D, 1], F32)
            nc.sync.dma_start(out=bias_c[:], in_=bias_in[:, :])
            # x chunks as separate tiles so phase-1 starts on chunk 0
            xtc = []
            for ci in range(NXC):
                xc = cp.tile([P, XCW], FP8)
                nc.sync.dma_start(
                    out=xc[:], in_=xt_in[:, ci * XCW : (ci + 1) * XCW]
                )
                xtc.append(xc)
            # x_own on the Act HWDGE ring (never stalls the aT stream)
            xowt = cp.tile([P, RPC], BF16)
            nc.scalar.dma_start(out=xowt[:], in_=xo_in[:, :])

            # ---- phase 1: he_all = x @ W.T -> fp8, SBUF-resident ----
            he8 = cp.tile([P, NT * D], FP8)
            he8_v = he8[:].rearrange("p (t f) -> p t f", f=D)
            TPX = XCW // P  # tiles per x chunk (20)
            for q in range(NT // 4):
                ci, qi = divmod(q, TPX // 4)
                xc = xtc[ci]
                ps = p1p.tile([P, 512], F32, space="PSUM", tag="ph1")
                for i in range(4):
                    t = 4 * qi + i
                    nc.tensor.matmul(
                        out=ps[:, i * P : (i + 1) * P],
                        lhsT=xc[:, t * P : (t + 1) * P], rhs=wsb[:],
                        start=True, stop=True, skip_group_check=True,
                    )
                if q % 2 == 0:
                    nc.vector.tensor_copy(
                        out=he8[:, q * 512 : (q + 1) * 512], in_=ps[:]
                    )
                else:
                    nc.scalar.activation(
                        out=he8[:, q * 512 : (q + 1) * 512], in_=ps[:],
                        func=AF.Copy,
                    )

            # ---- phase 2: fp8 DoubleRow aggregation, 512 dst cols/chain ----
            # hpsT[f, j] = sum_src he[src, f] * aT[src, j]
            hb = cp.tile([P, RPC], BF16)   # staged h rows, one output DMA
            goff = 0
            for gi, (c0, cw) in enumerate(GROUPS):
                hps = agp.tile([P, 512], F32, space="PSUM", tag="hps")
                for w in range(NW):
                    at_sb = atp.tile([P, WT * 512], FP8, tag="at")
                    nc.sync.dma_start(
                        out=at_sb[:, 0 : WT * cw],
                        in_=at_in[:, goff + w * WT * cw : goff + (w + 1) * WT * cw],
                    )
                    at_v = at_sb[:, 0 : WT * cw].rearrange("p (t c) -> p t c", c=cw)
                    for u in range(WT // 2):
                        nc.tensor.matmul(
                            out=hps[:, 0:cw],
                            lhsT=he8_v[:, w * WT + 2 * u : w * WT + 2 * u + 2, :],
                            rhs=at_v[:, 2 * u : 2 * u + 2, :],
                            start=(w == 0 and u == 0),
                            stop=(w == NW - 1 and u == WT // 2 - 1),
                            perf_mode=PM.DoubleRow,
                        )
                goff += NT * cw
                # h = Lrelu(agg + bias, alpha=0.02) + x_own  ([f, dst])
                h2 = wp.tile([P, 512], F32, tag="h2")
                nc.scalar.activation(
                    out=h2[:, 0:cw], in_=hps[:, 0:cw], func=AF.Lrelu,
                    bias=bias_c[:], alpha=0.02,
                )
                nc.vector.tensor_tensor(
                    out=hb[:, c0 : c0 + cw], in0=h2[:, 0:cw],
                    in1=xowt[:, c0 : c0 + cw], op=ALU.add,
                )
            nc.sync.dma_start(out=hout[:, :], in_=hb[:, :])

    nc.finalize()
    return nc


def build_kernel_b():
    nc = bacc.Bacc("TRN2", target_bir_lowering=False)
    gw_in = nc.declare_dram_parameter("gw", [P, GWW], BF16, isOutput=False)
    pred = nc.declare_dram_parameter("predr", [SPC * P, BAND], BF16, isOutput=True)

    with tile.TileContext(nc) as tc:
        with (
            tc.tile_pool(name="const", bufs=1) as cp,
            tc.tile_pool(name="mm", bufs=4, space="PSUM") as mp,
            tc.tile_pool(name="stage", bufs=3) as sp,
        ):
            # two chunk tiles: stripe 0's whole band is in gA, so its matmuls
            # start as soon as the first 1.34MB lands.
            gA = cp.tile([P, BAND], BF16)
            nc.sync.dma_start(out=gA[:], in_=gw_in[:, 0:BAND])
            gB = cp.tile([P, GWW - BAND], BF16)
            nc.sync.dma_start(out=gB[:], in_=gw_in[:, BAND:GWW])

            for l in range(SPC):
                lhs = gA[:, l * P : (l + 1) * P]
                # stripe band = gA[l*128 : BAND) then gB[0 : l*128), cut into
                # <=512-wide pieces aligned to the 512 grid of the band.
                pieces = []  # (pos, tile, src_off, width)
                pos = 0
                for tl, soff, w in ((gA, l * P, BAND - l * P), (gB, 0, l * P)):
                    done = 0
                    while done < w:
                        take = min(512 - pos % 512, w - done)
                        pieces.append((pos, tl, soff + done, take))
                        pos += take
                        done += take
                stage = sp.tile([P, BAND], BF16, tag="stage")
                for m in range(6):
                    lo, hi = m * 1024, min((m + 1) * 1024, BAND)
                    ps = mp.tile([P, 1024], F32, space="PSUM", tag="mm")
                    for ppos, tl, soff, take in pieces:
                        if lo <= ppos < hi:
                            nc.tensor.matmul(
                                out=ps[:, ppos - lo : ppos - lo + take],
                                lhsT=lhs,
                                rhs=tl[:, soff : soff + take],
                                start=True, stop=True, skip_group_check=True,
                            )
                    if (m + l) % 2 == 0:
                        nc.vector.tensor_copy(
                            out=stage[:, lo:hi], in_=ps[:, 0 : hi - lo]
                        )
                    else:
                        nc.scalar.activation(
                            out=stage[:, lo:hi], in_=ps[:, 0 : hi - lo],
                            func=AF.Copy,
                        )
                    if m == 2:   # first 3 chunks staged -> write first half
                        nc.sync.dma_start(
                            out=pred[l * P : (l + 1) * P, 0:3072],
                            in_=stage[:, 0:3072],
                        )
                nc.sync.dma_start(
                    out=pred[l * P : (l + 1) * P, 3072:BAND],
                    in_=stage[:, 3072:BAND],
                )

    nc.finalize()
    return nc


def _prep(x, edge_index, W, att_src, att_dst, bias):
    x = np.asarray(x, dtype=np.float32)
    edge_index = np.asarray(edge_index)
    W = np.asarray(W, dtype=np.float32)
    att_src = np.asarray(att_src, dtype=np.float32).reshape(D)
    att_dst = np.asarray(att_dst, dtype=np.float32).reshape(D)
    bias = np.asarray(bias, dtype=np.float32).reshape(D)

    n = x.shape[0]
    loops = np.arange(n, dtype=np.int64)
    src = np.concatenate([edge_index[0], loops]).astype(np.int64)
    dst = np.concatenate([edge_index[1], loops]).astype(np.int64)

    # exact host softmax (matches reference: leaky 0.2, segment max, +EPS)
    v_src = W.T @ att_src
    v_dst = W.T @ att_dst
    a_src = (x @ v_src).astype(np.float64)
    a_dst = (x @ v_dst).astype(np.float64)
    e = a_src[src] + a_dst[dst]
    e = np.where(e > 0, e, 0.2 * e)
    e_max = np.full(n, -np.inf)
    np.maximum.at(e_max, dst, e)
    e_max = np.where(np.isfinite(e_max), e_max, 0.0)
    e_exp = np.exp(e - e_max[dst])
    den = np.zeros(n)
    np.add.at(den, dst, e_exp)
    alpha_e = (e_exp / (den[dst] + EPS)).astype(np.float32)

    # dense alpha-weighted adjacency, transposed: aT[src, dst]
    aT = np.zeros((NPAD, NPAD), dtype=np.float32)
    np.add.at(aT, (src, dst), alpha_e)       # duplicates sum
    aT = aT.astype(FP8NP)

    x_pad = np.zeros((NPAD, D), dtype=np.float32)
    x_pad[:n] = x
    xT = np.ascontiguousarray(x_pad.T.astype(FP8NP))
    wT = np.ascontiguousarray(W.T.astype(BF16NP))
    xoT = np.ascontiguousarray(x_pad.T.astype(BF16NP))
    return xT, wT, bias.reshape(D, 1), aT, xoT


def _pack_at(aT_core):
    """[NPAD, RPC] fp8 -> [P, NT*RPC] with cols ordered (group, tile, col)
    so each (group, window) DMA slice is fully contiguous per partition."""
    parts = []
    for c0, cw in GROUPS:
        blk = aT_core[:, c0 : c0 + cw].reshape(NT, P, cw)
        parts.append(blk.transpose(1, 0, 2).reshape(P, NT * cw))
    return np.ascontiguousarray(np.concatenate(parts, axis=1))


def kernel(x, edge_index, W, att_src, att_dst, bias, _trace=False):
    xT, wT, bias_c, aT, xpT = _prep(x, edge_index, W, att_src, att_dst, bias)

    nc_a = build_kernel_a()
    in_maps_a = []
    for c in range(NC):
        in_maps_a.append(
            {
                "xT": xT,
                "wT": wT,
                "biasc": bias_c,
                "aTp": _pack_at(aT[:, c * RPC : (c + 1) * RPC]),
                "xownT": np.ascontiguousarray(xpT[:, c * RPC : (c + 1) * RPC]),
            }
        )
    res_a = run_bass_kernel_spmd(nc_a, in_maps_a, list(range(NC)), trace=_trace)
    ra = res_a.results

    hT = np.concatenate(
        [ra[c]["houtT"].astype(np.float32) for c in range(NC)], axis=1
    )  # [D, NPAD] (bf16 values)

    ss = float(np.sum(hT[:, :N].astype(np.float64) ** 2))
    gT = (hT / np.sqrt(ss)).astype(BF16NP)  # [D, NPAD]

    idx = np.arange(GWW)
    nc_b = build_kernel_b()
    in_maps_b = []
    for c in range(NC):
        cols = (c * RPC + idx) % NPAD
        in_maps_b.append({"gw": np.ascontiguousarray(gT[:, cols])})
    res_b = run_bass_kernel_spmd(nc_b, in_maps_b, list(range(NC)), trace=_trace)
    rb = res_b.results

    # assemble: stripe r owns col-blocks r..r+40 (mod 80); mirror the band.
    predp = np.empty((NPAD, NPAD), dtype=BF16NP)
    bidx = np.arange(BAND)
    pidx = np.arange(P)
    for c in range(NC):
        band = rb[c]["predr"]  # [1280, 5248] bf16
        for l in range(SPC):
            r = c * SPC + l
            rows = slice(r * P, (r + 1) * P)
            cols = (r * P + bidx) % NPAD
            blk = band[l * P : (l + 1) * P, :]
            predp[rows, cols] = blk
            predp[cols[:, None], (r * P + pidx)[None, :]] = blk.T
    pred = predp[:N, :N].astype(np.float32)

    kernel.last_results = (("A", res_a), ("B", res_b))
    return pred
